# revision 5
# baseline (speedup 1.0000x reference)
"""Trainium2 Bass kernel for nn_Conv_SNU_Network_classification.

SNU network: per time step t (20 steps):
  c   = conv2d(x_t, Wc)  (1->6 ch, 10x10, VALID, 64x64 -> 55x55)
  s1  = relu(c + 0.8*s1*(1-y1));  y1 = sigmoid(s1 + bc)
  h   = maxpool2(y1) -> [B, 4374]
  s2  = relu(h @ W2 + 0.8*s2*(1-y2));  y2 = sigmoid(s2 + b2)
Outputs: loss / m / out_rec / acc  (cross-entropy over m = mean_t y2).

Strategy (pure data parallelism, 16 images per core x 8 cores):
  - conv as 30 PE matmuls/step: block-Toeplitz stationary [128,112]
    (row-pair taps x (channel-pair, 55 j-positions)), rhs = transposed
    image row-pairs, PSUM-accumulated over 5 tap row-pairs.
  - leak term 0.8*s1*(1-y1) added into the same PSUM via an extra
    matmul with a 0.8*I stationary (u = s1*(1-y1) computed on GPSIMD).
  - relu/sigmoid on ScalarE (per-partition bias APs), pools as strided
    min on DVE (y1c = 1-y1 is carried, so maxpool(y1) = 1-minpool(y1c)),
    dense layer as 81 tiny PSUM-accumulated matmuls, s2 recurrence on
    [2,16] tiles.
  - host does the final (tiny) softmax/loss/acc reduction in numpy.
"""

import numpy as np

import concourse.bass as bass
import concourse.mybir as mybir
import concourse.tile as tile
from concourse.bass_utils import run_bass_kernel_spmd
from concourse.vector_clock import ScopedClock

F32 = mybir.dt.float32
N_CORES = 8
B = 128
B_LOC = B // N_CORES          # 16 images per core
T = 20
LEAK = 0.8
IMG = 64                      # image height/width
CO = 55                       # conv output height/width
PO = 27                       # pooled height/width
NE, NO = 448, 432             # even/odd conv-psum free sizes (16 img x 28/27 rows)

# ---------------------------------------------------------------------------
# Workarounds: this walrus build rejects >1 sync-wait per instruction.
# ---------------------------------------------------------------------------

_patched = [False]


def _patch_tile_drain():
    if _patched[0]:
        return
    _patched[0] = True

    def _drain_and_barrier(self, tick_clock, wait_clock):
        carrier = self.nc.sync.nop(nofuse=True, hint="tile_tail_wait_carrier")
        wait_clock.add_sem_waits(
            carrier.ins, ScopedClock({None: tick_clock.global_clock})
        )
        waits = list(carrier.ins.sync_info.on_wait)
        ups = list(carrier.ins.sync_info.on_update)
        carrier.ins.sync_info = mybir.SyncInfo(on_wait=waits[:1], on_update=ups)
        for w in waits[1:]:
            n = self.nc.sync.nop(nofuse=True, hint="tile_tail_wait")
            n.ins.sync_info = mybir.SyncInfo(on_wait=[w], on_update=[])
        self.nc.sync.drain()
        self.nc.all_engine_barrier(sem_only=True)
        assert self.sems is not None
        popped = self.nc._tile_sem_poison_stack.pop()
        assert popped is self._sem_poison
        self.nc.clear_and_free_semaphores(list(self.sems.allocated().values()))
        self.nc.all_engine_barrier(sem_only=True)

    tile.TileContext._drain_and_barrier = _drain_and_barrier


_uid = [0]


def _split_multiwaits(nc):
    """Split any instruction carrying >1 sync-wait into preceding
    single-wait NoOps on the same engine (same program order => same
    semantics)."""
    n_split = 0
    for f in nc.m.functions:
        for blk in f.blocks:
            insts = list(blk.instructions)
            out = []
            changed = False
            for ins in insts:
                si = ins.sync_info
                if si is not None and len(si.on_wait) > 1:
                    changed = True
                    n_split += 1
                    waits = list(si.on_wait)
                    for w in waits[:-1]:
                        _uid[0] += 1
                        nop = mybir.InstNoOp(name=f"mwsplit-{_uid[0]}")
                        nop.engine = ins.engine
                        nop.sync_info = mybir.SyncInfo(on_wait=[w], on_update=[])
                        out.append(nop)
                    ins.sync_info = mybir.SyncInfo(
                        on_wait=[waits[-1]], on_update=list(si.on_update)
                    )
                out.append(ins)
            if changed:
                blk.instructions = out
    return n_split


# ---------------------------------------------------------------------------
# Host-side constant construction
# ---------------------------------------------------------------------------


def _build_consts(Wc, bc, W2, b2):
    Wc = np.asarray(Wc, np.float32)
    bc = np.asarray(bc, np.float32)
    W2 = np.asarray(W2, np.float32)
    b2 = np.asarray(b2, np.float32)

    # Conv Toeplitz stationaries: lt[cp*5+p, 64*d+u, 56*o'+j]
    lt = np.zeros((15, 128, 128), np.float32)
    j = np.arange(CO)
    kj = np.arange(10)
    u = j[None, :] + kj[:, None]          # [10, 55]
    # column layout: m = 56*o' + (j//2 if j even else 28 + j//2), so the
    # 2x2 pool's j-pairs live in two step-1 partition blocks
    mcol = np.where(j % 2 == 0, j // 2, 32 + j // 2)
    for cp in range(3):
        for p in range(5):
            s = cp * 5 + p
            for op in range(2):
                o = 2 * cp + op
                for d in range(2):
                    ki = 2 * p + d
                    lt[s, 64 * d + u, 64 * op + mcol[None, :]] = Wc[o, 0, ki, :][
                        :, None
                    ]
    i8 = (LEAK * np.eye(128)).astype(np.float32)

    # Dense stationaries: w2d[cp, i', m=(o'*28+j'), cls]
    w2d = np.zeros((3, PO, 128, 2), np.float32)
    for cp in range(3):
        for op in range(2):
            ch = 2 * cp + op
            blk = W2[ch * PO * PO : (ch + 1) * PO * PO].reshape(PO, PO, 2)
            w2d[cp, :, op * 64 : op * 64 + PO, :] = blk
    nbc = np.zeros((128, 3), np.float32)
    q = np.arange(128)
    for cp in range(3):
        nbc[:, cp] = -bc[2 * cp + q // 64]
    b2v = b2.reshape(2, 1).astype(np.float32)
    c2v = W2.sum(0).reshape(2, 1).astype(np.float32)
    ident = np.eye(128, dtype=np.float32)
    return dict(lt=lt, i8=i8, w2d=w2d, nbc=nbc, b2v=b2v, c2v=c2v, ident=ident)


# ---------------------------------------------------------------------------
# Device program
# ---------------------------------------------------------------------------


def _build_program():
    _patch_tile_drain()
    nc = bass.Bass()

    xin = nc.dram_tensor("xin", [B_LOC * IMG * IMG * T], F32, kind="ExternalInput")
    lt_d = nc.dram_tensor("lt", [15, 128, 128], F32, kind="ExternalInput")
    i8_d = nc.dram_tensor("i8", [128, 128], F32, kind="ExternalInput")
    w2d_d = nc.dram_tensor("w2d", [3, PO, 128, 2], F32, kind="ExternalInput")
    nbc_d = nc.dram_tensor("nbc", [128, 3], F32, kind="ExternalInput")
    b2v_d = nc.dram_tensor("b2v", [2, 1], F32, kind="ExternalInput")
    c2v_d = nc.dram_tensor("c2v", [2, 1], F32, kind="ExternalInput")
    id_d = nc.dram_tensor("ident", [128, 128], F32, kind="ExternalInput")
    orec = nc.dram_tensor("orec", [B_LOC, T + 1, 2], F32, kind="ExternalOutput")

    with tile.TileContext(nc) as tc:
        with (
            tc.tile_pool(name="pconst", bufs=1) as pconst,
            tc.tile_pool(name="pstate", bufs=1) as pstate,
            tc.tile_pool(name="pxt", bufs=3) as pxt,
            tc.tile_pool(name="pu", bufs=2) as pu,
            tc.tile_pool(name="ptmp", bufs=2) as ptmp,
            tc.tile_pool(name="ptp", bufs=1, space="PSUM") as ptp,
            tc.tile_pool(name="pcv", bufs=3, space="PSUM") as pcv,
            tc.tile_pool(name="pdn", bufs=2, space="PSUM") as pdn,
        ):
            # ---- constants to SBUF ----
            X_raw = pconst.tile([128, 576 * T], F32)
            LT = pconst.tile([128, 15 * 128], F32)
            I8 = pconst.tile([128, 128], F32)
            W2D = pconst.tile([128, 3 * PO * 2], F32)
            NBC = pconst.tile([128, 3], F32)
            B2V = pconst.tile([2, 1], F32)
            C2V = pconst.tile([2, 1], F32)
            IDT = pconst.tile([128, 128], F32)

            xin_v = xin[:].rearrange("(p f) -> p f", p=128)   # [128, 10240]
            nc.sync.dma_start(X_raw[:, 0 : 512 * T], xin_v)
            nc.vector.memset(X_raw[:, 512 * T :], 0.0)
            nc.sync.dma_start(
                X_raw[0:127, 512 * T : 576 * T], xin_v[1:128, 0 : 64 * T]
            )
            nc.sync.dma_start(
                LT[:].rearrange("k (s m) -> k s m", s=15),
                lt_d[:].transpose([1, 0, 2]),
            )
            nc.sync.dma_start(I8[:], i8_d[:])
            nc.sync.dma_start(
                W2D[:].rearrange("k (c i l) -> k c i l", c=3, i=PO),
                w2d_d[:].transpose([2, 0, 1, 3]),
            )
            nc.sync.dma_start(NBC[:], nbc_d[:])
            nc.sync.dma_start(B2V[:], b2v_d[:])
            nc.sync.dma_start(C2V[:], c2v_d[:])
            nc.sync.dma_start(IDT[:], id_d[:])

            # ---- persistent state ----
            s1e = pstate.tile([128, 3 * NE], F32)
            s1o = pstate.tile([128, 3 * NO], F32)
            y1e = pstate.tile([128, 3 * NE], F32)
            y1o = pstate.tile([128, 3 * NO], F32)
            s2 = pstate.tile([2, B_LOC], F32)
            y2c8 = pstate.tile([2, B_LOC], F32)
            Y2H = pstate.tile([2, (T + 1) * B_LOC], F32)

            nc.vector.memset(s1e[:], 0.0)
            nc.vector.memset(s1o[:], 0.0)
            nc.vector.memset(y1e[:], 1.0)
            nc.vector.memset(y1o[:], 1.0)
            nc.vector.memset(s2[:], 0.0)
            nc.vector.memset(y2c8[:], LEAK)
            nc.vector.memset(Y2H[:], 0.0)

            Xr = X_raw[:].rearrange("p (pix t) -> p pix t", t=T)  # [128,576,20]

            for t in range(T):
                # u = s1 * y1c from previous step (gpsimd, 2-input mult)
                ue = pu.tile([128, 3 * NE], F32, tag="ue")
                uo = pu.tile([128, 3 * NO], F32, tag="uo")
                nc.gpsimd.tensor_tensor(
                    out=ue[:], in0=s1e[:], in1=y1e[:], op=mybir.AluOpType.mult
                )
                nc.gpsimd.tensor_tensor(
                    out=uo[:], in0=s1o[:], in1=y1o[:], op=mybir.AluOpType.mult
                )

                # ---- transpose x_t into row-pair layout ----
                psumT = ptp.tile([128, 1024], F32, tag="psumT")
                for c8 in range(8):
                    pix_off = 128 * (c8 % 4) + (64 if c8 >= 4 else 0)
                    nc.tensor.transpose(
                        psumT[:, 128 * c8 : 128 * (c8 + 1)],
                        Xr[:, pix_off : pix_off + 128, t],
                        IDT[:],
                    )
                XT = pxt.tile([128, 1024], F32, tag="XT")
                P5 = psumT[:].rearrange("q (e c i b) -> q e c i b", e=2, c=4, i=16)
                X5 = XT[:].rearrange("q (i e b c) -> q i e b c", i=16, e=2, b=8)
                for e in range(2):
                    nc.vector.tensor_copy(
                        X5[:, :, e], P5[:, e].transpose([0, 2, 3, 1])
                    )
                XTv = XT[:].rearrange("q (i g k) -> q i g k", i=16, g=2)

                # ---- conv + leak + relu + sigmoid, per (channel-pair, parity) ----
                for cp in range(3):
                    for par, (NP, ni) in enumerate([(NE, 28), (NO, PO)]):
                        ps = pcv.tile([128, NP], F32, tag="cpsum")
                        for p in range(5):
                            nc.tensor.matmul(
                                ps[:],
                                LT[:, (cp * 5 + p) * 128 : (cp * 5 + p + 1) * 128],
                                XTv[:, :, par, p : p + ni],
                                start=(p == 0),
                                stop=False,
                            )
                        u_t, s1_t, y1_t = (
                            (ue, s1e, y1e) if par == 0 else (uo, s1o, y1o)
                        )
                        sl = slice(cp * NP, (cp + 1) * NP)
                        nc.tensor.matmul(
                            ps[:],
                            I8[:],
                            u_t[:, sl],
                            start=False,
                            stop=True,
                            skip_group_check=True,
                        )
                        nc.scalar.activation(
                            s1_t[:, sl], ps[:], mybir.ActivationFunctionType.Relu
                        )
                        nc.scalar.activation(
                            y1_t[:, sl],
                            s1_t[:, sl],
                            mybir.ActivationFunctionType.Sigmoid,
                            bias=NBC[:, cp : cp + 1],
                            scale=-1.0,
                        )

                # ---- pools (min of y1c == 1 - maxpool(y1)) ----
                pj_e = ptmp.tile([128, 3 * NE], F32, tag="pje")
                pj_o = ptmp.tile([128, 3 * NO], F32, tag="pjo")
                hm = ptmp.tile([128, 3 * 16 * PO], F32, tag="hm")
                scr_e = ptmp.tile([128, 3 * NE], F32, tag="scre")
                scr_o = ptmp.tile([128, 3 * NO], F32, tag="scro")
                nc.gpsimd.memset(pj_e[:], 0.0)
                nc.gpsimd.memset(pj_o[:], 0.0)
                for op in range(2):
                    # shift the odd-j block down 32 partitions via DMA so the
                    # DVE min sees equal base partitions on both inputs
                    nc.sync.dma_start(
                        scr_e[64 * op : 64 * op + 32],
                        y1e[64 * op + 32 : 64 * (op + 1)],
                    )
                    nc.sync.dma_start(
                        scr_o[64 * op : 64 * op + 32],
                        y1o[64 * op + 32 : 64 * (op + 1)],
                    )
                    nc.vector.tensor_tensor(
                        out=pj_e[64 * op : 64 * op + 32],
                        in0=y1e[64 * op : 64 * op + 32],
                        in1=scr_e[64 * op : 64 * op + 32],
                        op=mybir.AluOpType.min,
                    )
                    nc.vector.tensor_tensor(
                        out=pj_o[64 * op : 64 * op + 32],
                        in0=y1o[64 * op : 64 * op + 32],
                        in1=scr_o[64 * op : 64 * op + 32],
                        op=mybir.AluOpType.min,
                    )
                pev = pj_e[:].rearrange("q (c i m) -> q c i m", c=3, i=16)
                pov = pj_o[:].rearrange("q (c i m) -> q c i m", c=3, i=16)
                hmv = hm[:].rearrange("q (c i m) -> q c i m", c=3, i=16)
                nc.vector.tensor_tensor(
                    out=hmv[:, :, :, :],
                    in0=pev[:, :, :, 0:PO],
                    in1=pov[:, :, :, :],
                    op=mybir.AluOpType.min,
                )

                # ---- dense: d = sum W2 * minpool, accumulated on PE ----
                dps = pdn.tile([2, B_LOC], F32, tag="dps")
                W2Dv = W2D[:].rearrange("k (c i l) -> k c i l", c=3, i=PO)
                n_mm = 3 * PO
                idx = 0
                for cp in range(3):
                    for ip in range(PO):
                        nc.tensor.matmul(
                            dps[:],
                            W2Dv[:, cp, ip],
                            hmv[:, cp, :, ip],
                            start=(idx == 0),
                            stop=(idx == n_mm - 1),
                        )
                        idx += 1

                # ---- s2 recurrence (tiny) ----
                pre = ptmp.tile([2, B_LOC], F32, tag="pre")
                v = ptmp.tile([2, B_LOC], F32, tag="v")
                s2t = ptmp.tile([2, B_LOC], F32, tag="s2t")
                # pre = c2v - d
                nc.vector.tensor_scalar(
                    out=pre[:],
                    in0=dps[:],
                    scalar1=C2V[:, 0:1],
                    scalar2=-1.0,
                    op0=mybir.AluOpType.subtract,
                    op1=mybir.AluOpType.mult,
                )
                nc.vector.tensor_tensor(
                    out=v[:], in0=s2[:], in1=y2c8[:], op=mybir.AluOpType.mult
                )
                nc.vector.tensor_tensor(
                    out=s2t[:], in0=pre[:], in1=v[:], op=mybir.AluOpType.add
                )
                nc.vector.tensor_scalar_max(s2[:], s2t[:], 0.0)
                ysl = Y2H[:, (t + 1) * B_LOC : (t + 2) * B_LOC]
                nc.scalar.activation(
                    ysl,
                    s2[:],
                    mybir.ActivationFunctionType.Sigmoid,
                    bias=B2V[:, 0:1],
                    scale=1.0,
                )
                nc.vector.tensor_scalar(
                    out=y2c8[:],
                    in0=ysl,
                    scalar1=-LEAK,
                    scalar2=LEAK,
                    op0=mybir.AluOpType.mult,
                    op1=mybir.AluOpType.add,
                )

            # ---- write out y2 history: orec[i, s, c] <- Y2H[c, (s, i)] ----
            orec_csi = orec[:].transpose([2, 1, 0])  # (c, s, i)
            for s in range(T + 1):
                nc.sync.dma_start(
                    orec_csi[:, s],
                    Y2H[:, s * B_LOC : (s + 1) * B_LOC],
                )

    _split_multiwaits(nc)
    return nc


_PROGRAM = [None]


def _get_program():
    if _PROGRAM[0] is None:
        _PROGRAM[0] = _build_program()
    return _PROGRAM[0]


# ---------------------------------------------------------------------------
# Entry point
# ---------------------------------------------------------------------------


def kernel(x, y, Wc, bc, W2, b2, _collect_results=None):
    x = np.asarray(x)
    y_np = np.asarray(y)
    consts = _build_consts(Wc, bc, W2, b2)

    nc = _get_program()
    in_maps = []
    for c in range(N_CORES):
        xs = np.ascontiguousarray(
            x[c * B_LOC : (c + 1) * B_LOC], dtype=np.float32
        ).reshape(-1)
        in_maps.append({"xin": xs, **consts})
    res = run_bass_kernel_spmd(nc, in_maps, core_ids=list(range(N_CORES)))
    if _collect_results is not None:
        _collect_results.append(res)

    out_rec = np.concatenate(
        [np.asarray(res.results[c]["orec"], np.float32) for c in range(N_CORES)],
        axis=0,
    )  # [128, 21, 2]

    m = (out_rec.sum(axis=1) / 20.0).astype(np.float32)
    mm = m - m.max(axis=1, keepdims=True)
    logp = (mm - np.log(np.exp(mm).sum(axis=1, keepdims=True))).astype(np.float32)
    loss = np.float32(-logp[np.arange(B), y_np].mean())
    acc = np.float32((m.argmax(axis=1) == y_np).mean())
    return (loss, m, out_rec, acc)
